# revision 39
# baseline (speedup 1.0000x reference)
"""MoE stacked-expert linear: y[e] = x @ W[e].T for 8 experts.

Full-input contract: kernel(x=[4,2048,4096] f32, W=[8,64,4096] f32) ->
tuple of 8 arrays [4,2048,64] f32 (matches the reference's return pytree).

Strategy: one GEMM [T=8192, D=4096] @ [D, E*R=512], token-parallel across
8 NeuronCores (1024 tokens each).  The host packs, per 128-row K-chunk,
the x-chunk [128, 1024] and the W-chunk [128, 512] side by side into one
DRAM tensor, so each K-step needs exactly ONE contiguous DMA (the fp32
self-loading matmul's LDWEIGHTS slot only tolerates a single semaphore
wait).  On-chip: 8 PSUM banks (one per 128-token tile) accumulate over
the 32 K-chunks; DVE drains PSUM -> SBUF -> DRAM.
"""

import numpy as np

import concourse.bass as bass
import concourse.mybir as mybir
import concourse.tile as tile
from concourse.bass_utils import run_bass_kernel_spmd

N_CORES = 8
B, S, D = 4, 2048, 4096
E, R = 8, 64
T = B * S            # 8192 tokens
TPC = T // N_CORES   # 1024 tokens per core
ER = E * R           # 512 output features
P = 128              # partition / tile edge
NK = D // P          # 32 contraction chunks
NM = TPC // P        # 8 token tiles per core
C = TPC + ER         # packed columns per chunk (x | w)

MODE = "fp32r"       # "fp32" | "fp32r" | "bf16" | "fp16" | "bf16x3"
CPD = 1              # k-chunks per DMA
N_WARMUP = 20        # dummy N=512 matmuls to warm the PE clock gate

_nc_cache = {}

_MODE_DT = {
    "fp32": "float32",
    "fp32r": "float32r",
    "bf16": "bfloat16",
    "fp16": "float16",
    "bf16x3": "bfloat16",
}


class _LeanTailTileContext(tile.TileContext):
    """TileContext with a cheaper exit: keep the SP drain (gates NEFF end
    on output-DMA completion) and one all-engine barrier, but skip the
    semaphore clear pass and second barrier (~5-7us of EVSEM butterfly).
    Safe for single-execution NEFFs; sem state is re-initialized at load."""

    def _drain_and_barrier(self, tick_clock, wait_clock):
        from concourse.vector_clock import ScopedClock

        drain_inst = self.nc.sync.drain()
        wait_clock.add_sem_waits(
            drain_inst.ins, ScopedClock({None: tick_clock.global_clock})
        )
        self.nc.all_engine_barrier()
        popped = self.nc._tile_sem_poison_stack.pop()
        assert popped is self._sem_poison


def _build(mode):
    nc = bass.Bass()
    f32 = mybir.dt.float32
    in_dt = getattr(mybir.dt, _MODE_DT[mode])

    ncol = 2 * C if mode == "bf16x3" else C
    # Group-major DRAM layout: CPD k-chunks are interleaved so each DMA
    # group is one contiguous block AND each partition's slice within the
    # group is contiguous (CPD*ncol elements) -> fat DMA descriptors.
    pk = nc.dram_tensor("pk", [NK // CPD, P, CPD, ncol], in_dt,
                        kind="ExternalInput")
    out = nc.dram_tensor("out", [TPC, ER], f32, kind="ExternalOutput")

    with _LeanTailTileContext(nc) as tc:
        # Output DMAs go via SWDGE (gpsimd) so they don't perturb the
        # HWDGE queue rotation of the stream DMAs; opool bufs=NM so output
        # staging tiles are never reused (no WAR wait on the DVE copies).
        # Multi-wait instructions that remain are fixed up by
        # _legalize_waits.
        with (
            tc.tile_pool(name="x", bufs=8) as xpool,
            tc.tile_pool(name="o", bufs=NM) as opool,
            tc.tile_pool(name="ps", bufs=NM, space="PSUM") as pspool,
        ):
            psums = [pspool.tile([P, ER], f32, tag="ps", name=f"ps{m}")
                     for m in range(NM)]
            for kd in range(NK // CPD):
                pk_sb = xpool.tile([P, CPD, ncol], in_dt, tag="x",
                                   name=f"pk{kd}")
                nc.sync.dma_start(pk_sb[:], pk[kd])
                for kk in range(CPD):
                    k = kd * CPD + kk
                    first = k == 0
                    last = k == NK - 1
                    sb = pk_sb[:, kk, :]
                    for m in range(NM):
                        msl = slice(m * P, (m + 1) * P)
                        if mode == "bf16x3":
                            # x @ w ~= xh@wh + xl@wh + xh@wl  (drop xl@wl)
                            nc.tensor.matmul(psums[m][:], sb[:, msl],
                                             sb[:, TPC:C],
                                             start=first, stop=False)
                            nc.tensor.matmul(psums[m][:],
                                             sb[:, C + m * P:C + (m + 1) * P],
                                             sb[:, TPC:C],
                                             start=False, stop=False)
                            nc.tensor.matmul(psums[m][:], sb[:, msl],
                                             sb[:, C + TPC:],
                                             start=False, stop=last)
                        else:
                            nc.tensor.matmul(psums[m][:], sb[:, msl],
                                             sb[:, TPC:C],
                                             start=first, stop=last)
            for m in range(NM):
                o_sb = opool.tile([P, ER], f32, tag="o", name=f"o{m}")
                # Alternate DVE / ScalarE so the PSUM drain isn't serialized
                # on one engine at the tail.
                if m % 2 == 0:
                    nc.vector.tensor_copy(o_sb[:], psums[m][:])
                else:
                    nc.scalar.copy(o_sb[:], psums[m][:])
                nc.gpsimd.dma_start(out[m * P:(m + 1) * P, :], o_sb[:])
    return nc


def _legalize_waits(nc):
    """Walrus on this target accepts at most one sync wait per hardware
    instruction; hoist extra waits onto same-engine EventSemaphore
    preludes (the sequencer honors them in program order)."""
    import json

    import bass_rust

    bir = json.loads(nc.to_json_bytes())
    n = 0
    for fn in bir["functions"]:
        for blk in fn["blocks"]:
            out = []
            for inst in blk["instructions"]:
                si = inst.get("sync_info")
                waits = (si or {}).get("on_wait") or []
                if len(waits) > 1:
                    for w in waits[:-1]:
                        n += 1
                        out.append({
                            "debug": inst.get("debug", 0),
                            "engine": inst["engine"],
                            "ins": [],
                            "outs": [],
                            "name": f"legwait{n}",
                            "opcode": "EventSemaphore",
                            "sync_info": {"on_update": [], "on_wait": [w]},
                        })
                    si["on_wait"] = waits[-1:]
                out.append(inst)
            blk["instructions"] = out
    if n:
        nc.m = bass_rust.module_from_json_bytes(json.dumps(bir).encode())
    return nc


def _get_nc(mode):
    if mode not in _nc_cache:
        _nc_cache[mode] = _legalize_waits(_build(mode))
    return _nc_cache[mode]


def _prep_inputs(x, W, mode):
    """Host-side packing: per core, [NK, P, ncol] with x|w (and lo halves)."""
    import ml_dtypes

    x3 = x.reshape(N_CORES, TPC, D)                    # token shards
    w2 = W.reshape(ER, D)
    ins = []
    for i in range(N_CORES):
        # [NK, P, TPC]: xT chunk rows; [NK, P, ER]: wT chunk rows
        xTc = np.ascontiguousarray(
            x3[i].T.reshape(NK, P, TPC))               # d-major
        wTc = np.ascontiguousarray(w2.T.reshape(NK, P, ER))
        if mode in ("fp32", "fp32r"):
            pk = np.concatenate([xTc, wTc], axis=2)
        elif mode == "bf16":
            pk = np.concatenate([xTc, wTc], axis=2).astype(ml_dtypes.bfloat16)
        elif mode == "fp16":
            pk = np.concatenate([xTc, wTc], axis=2).astype(np.float16)
        elif mode == "bf16x3":
            xh = xTc.astype(ml_dtypes.bfloat16)
            wh = wTc.astype(ml_dtypes.bfloat16)
            xl = (xTc - xh.astype(np.float32)).astype(ml_dtypes.bfloat16)
            wl = (wTc - wh.astype(np.float32)).astype(ml_dtypes.bfloat16)
            pk = np.concatenate([xh, wh, xl, wl], axis=2)
        else:
            raise ValueError(mode)
        # [NK, P, ncol] -> group-major [NK//CPD, P, CPD, ncol]
        nc_ = pk.shape[2]
        pkg = pk.reshape(NK // CPD, CPD, P, nc_).transpose(0, 2, 1, 3)
        ins.append({"pk": np.ascontiguousarray(pkg)})
    return ins


def _run(x, W, mode, trace=False, tmpdir=None):
    nc = _get_nc(mode)
    in_maps = _prep_inputs(x, W, mode)
    res = run_bass_kernel_spmd(nc, in_maps, core_ids=list(range(N_CORES)),
                               trace=trace, tmpdir=tmpdir)
    full = np.concatenate([res.results[i]["out"] for i in range(N_CORES)], axis=0)
    y = full.reshape(B, S, E, R).transpose(2, 0, 1, 3)   # [E, B, S, R]
    return tuple(np.ascontiguousarray(y[e]) for e in range(E)), res


def kernel(x, W):
    x = np.asarray(x, dtype=np.float32)
    W = np.asarray(W, dtype=np.float32)
    y, _ = _run(x, W, MODE)
    return y


# revision 43
# speedup vs baseline: 1.1210x; 1.1210x over previous
"""MoE stacked-expert linear: y[e] = x @ W[e].T for 8 experts.

Full-input contract: kernel(x=[4,2048,4096] f32, W=[8,64,4096] f32) ->
tuple of 8 arrays [4,2048,64] f32 (matches the reference's return pytree).

Strategy: one GEMM [T=8192, D=4096] @ [D, E*R=512], token-parallel across
8 NeuronCores (1024 tokens each).  The host packs, per 128-row K-chunk,
the x-chunk [128, 1024] and the W-chunk [128, 512] side by side into one
DRAM tensor, so each K-step needs exactly ONE contiguous DMA (the fp32
self-loading matmul's LDWEIGHTS slot only tolerates a single semaphore
wait).  On-chip: 8 PSUM banks (one per 128-token tile) accumulate over
the 32 K-chunks; DVE drains PSUM -> SBUF -> DRAM.
"""

import numpy as np

import concourse.bass as bass
import concourse.mybir as mybir
import concourse.tile as tile
from concourse.bass_utils import run_bass_kernel_spmd

N_CORES = 8
B, S, D = 4, 2048, 4096
E, R = 8, 64
T = B * S            # 8192 tokens
TPC = T // N_CORES   # 1024 tokens per core
ER = E * R           # 512 output features
P = 128              # partition / tile edge
NK = D // P          # 32 contraction chunks
NM = TPC // P        # 8 token tiles per core
C = TPC + ER         # packed columns per chunk (x | w)

MODE = "fp32r"       # "fp32" | "fp32r" | "bf16" | "fp16" | "bf16x3"
CPD = 1              # k-chunks per DMA

_nc_cache = {}

_MODE_DT = {
    "fp32": "float32",
    "fp32r": "float32r",
    "bf16": "bfloat16",
    "fp16": "float16",
    "bf16x3": "bfloat16",
}


class _LeanTailTileContext(tile.TileContext):
    """TileContext with a cheaper exit: keep the SP drain (gates NEFF end
    on output-DMA completion) and one all-engine barrier, but skip the
    semaphore clear pass and second barrier (~5-7us of EVSEM butterfly).
    Safe for single-execution NEFFs; sem state is re-initialized at load."""

    def _drain_and_barrier(self, tick_clock, wait_clock):
        from concourse.vector_clock import ScopedClock

        drain_inst = self.nc.sync.drain()
        wait_clock.add_sem_waits(
            drain_inst.ins, ScopedClock({None: tick_clock.global_clock})
        )
        popped = self.nc._tile_sem_poison_stack.pop()
        assert popped is self._sem_poison


def _build(mode):
    nc = bass.Bass()
    f32 = mybir.dt.float32
    in_dt = getattr(mybir.dt, _MODE_DT[mode])

    ncol = 2 * C if mode == "bf16x3" else C
    # Group-major DRAM layout: CPD k-chunks are interleaved so each DMA
    # group is one contiguous block AND each partition's slice within the
    # group is contiguous (CPD*ncol elements) -> fat DMA descriptors.
    pk = nc.dram_tensor("pk", [NK // CPD, P, CPD, ncol], in_dt,
                        kind="ExternalInput")
    out = nc.dram_tensor("out", [TPC, ER], f32, kind="ExternalOutput")

    with _LeanTailTileContext(nc) as tc:
        # Output DMAs go via SWDGE (gpsimd) so they don't perturb the
        # HWDGE queue rotation of the stream DMAs; opool bufs=NM so output
        # staging tiles are never reused (no WAR wait on the DVE copies).
        # Multi-wait instructions that remain are fixed up by
        # _legalize_waits.
        with (
            tc.tile_pool(name="x", bufs=8) as xpool,
            tc.tile_pool(name="o", bufs=NM) as opool,
            tc.tile_pool(name="ps", bufs=NM, space="PSUM") as pspool,
        ):
            psums = [pspool.tile([P, ER], f32, tag="ps", name=f"ps{m}")
                     for m in range(NM)]
            for kd in range(NK // CPD):
                pk_sb = xpool.tile([P, CPD, ncol], in_dt, tag="x",
                                   name=f"pk{kd}")
                nc.sync.dma_start(pk_sb[:], pk[kd])
                for kk in range(CPD):
                    k = kd * CPD + kk
                    first = k == 0
                    last = k == NK - 1
                    sb = pk_sb[:, kk, :]
                    for m in range(NM):
                        msl = slice(m * P, (m + 1) * P)
                        if mode == "bf16x3":
                            # x @ w ~= xh@wh + xl@wh + xh@wl  (drop xl@wl)
                            nc.tensor.matmul(psums[m][:], sb[:, msl],
                                             sb[:, TPC:C],
                                             start=first, stop=False)
                            nc.tensor.matmul(psums[m][:],
                                             sb[:, C + m * P:C + (m + 1) * P],
                                             sb[:, TPC:C],
                                             start=False, stop=False)
                            nc.tensor.matmul(psums[m][:], sb[:, msl],
                                             sb[:, C + TPC:],
                                             start=False, stop=last)
                        else:
                            nc.tensor.matmul(psums[m][:], sb[:, msl],
                                             sb[:, TPC:C],
                                             start=first, stop=last)
            for m in range(NM):
                o_sb = opool.tile([P, ER], f32, tag="o", name=f"o{m}")
                # Alternate DVE / ScalarE so the PSUM drain isn't serialized
                # on one engine, and alternate the out-DMA enqueues between
                # GpSimd (SWDGE) and the now-idle PE (HWDGE) for the same
                # reason (~650ns per enqueue).
                if m % 2 == 0:
                    nc.vector.tensor_copy(o_sb[:], psums[m][:])
                    nc.gpsimd.dma_start(out[m * P:(m + 1) * P, :], o_sb[:])
                else:
                    nc.scalar.copy(o_sb[:], psums[m][:])
                    nc.sync.dma_start(out[m * P:(m + 1) * P, :], o_sb[:])
    return nc


def _legalize_waits(nc):
    """Walrus on this target accepts at most one sync wait per hardware
    instruction; hoist extra waits onto same-engine EventSemaphore
    preludes (the sequencer honors them in program order)."""
    import json

    import bass_rust

    bir = json.loads(nc.to_json_bytes())
    n = 0
    for fn in bir["functions"]:
        for blk in fn["blocks"]:
            out = []
            for inst in blk["instructions"]:
                si = inst.get("sync_info")
                waits = (si or {}).get("on_wait") or []
                if len(waits) > 1:
                    for w in waits[:-1]:
                        n += 1
                        out.append({
                            "debug": inst.get("debug", 0),
                            "engine": inst["engine"],
                            "ins": [],
                            "outs": [],
                            "name": f"legwait{n}",
                            "opcode": "EventSemaphore",
                            "sync_info": {"on_update": [], "on_wait": [w]},
                        })
                    si["on_wait"] = waits[-1:]
                out.append(inst)
            blk["instructions"] = out
    if n:
        nc.m = bass_rust.module_from_json_bytes(json.dumps(bir).encode())
    return nc


def _get_nc(mode):
    if mode not in _nc_cache:
        _nc_cache[mode] = _legalize_waits(_build(mode))
    return _nc_cache[mode]


def _prep_inputs(x, W, mode):
    """Host-side packing: per core, [NK, P, ncol] with x|w (and lo halves)."""
    import ml_dtypes

    x3 = x.reshape(N_CORES, TPC, D)                    # token shards
    w2 = W.reshape(ER, D)
    ins = []
    for i in range(N_CORES):
        # [NK, P, TPC]: xT chunk rows; [NK, P, ER]: wT chunk rows
        xTc = np.ascontiguousarray(
            x3[i].T.reshape(NK, P, TPC))               # d-major
        wTc = np.ascontiguousarray(w2.T.reshape(NK, P, ER))
        if mode in ("fp32", "fp32r"):
            pk = np.concatenate([xTc, wTc], axis=2)
        elif mode == "bf16":
            pk = np.concatenate([xTc, wTc], axis=2).astype(ml_dtypes.bfloat16)
        elif mode == "fp16":
            pk = np.concatenate([xTc, wTc], axis=2).astype(np.float16)
        elif mode == "bf16x3":
            xh = xTc.astype(ml_dtypes.bfloat16)
            wh = wTc.astype(ml_dtypes.bfloat16)
            xl = (xTc - xh.astype(np.float32)).astype(ml_dtypes.bfloat16)
            wl = (wTc - wh.astype(np.float32)).astype(ml_dtypes.bfloat16)
            pk = np.concatenate([xh, wh, xl, wl], axis=2)
        else:
            raise ValueError(mode)
        # [NK, P, ncol] -> group-major [NK//CPD, P, CPD, ncol]
        nc_ = pk.shape[2]
        pkg = pk.reshape(NK // CPD, CPD, P, nc_).transpose(0, 2, 1, 3)
        ins.append({"pk": np.ascontiguousarray(pkg)})
    return ins


def _run(x, W, mode, trace=False, tmpdir=None):
    nc = _get_nc(mode)
    in_maps = _prep_inputs(x, W, mode)
    res = run_bass_kernel_spmd(nc, in_maps, core_ids=list(range(N_CORES)),
                               trace=trace, tmpdir=tmpdir)
    full = np.concatenate([res.results[i]["out"] for i in range(N_CORES)], axis=0)
    y = full.reshape(B, S, E, R).transpose(2, 0, 1, 3)   # [E, B, S, R]
    return tuple(np.ascontiguousarray(y[e]) for e in range(E)), res


def kernel(x, W):
    x = np.asarray(x, dtype=np.float32)
    W = np.asarray(W, dtype=np.float32)
    y, _ = _run(x, W, MODE)
    return y
